# revision 11
# baseline (speedup 1.0000x reference)
"""Trainium2 Bass kernel for a single-step 4-layer LSTM decoder with
Bahdanau attention and a 50k-vocab output projection, SPMD across 8
NeuronCores.

Distribution (hardcoded for B=32, S=2048, H=E=1024, V=50257, L=4):
  - Attention is data-parallel over batch (4 per core). The additive
    energy's hidden term is constant per row and cancels in softmax, and v
    folds through Wa_e on the host into one vector u, so the whole
    attention scan is independent of the LSTM and streams encoder_outputs
    exactly once (unnormalized softmax accumulated in PSUM via rank-1
    fp32r matmuls).
  - LSTM is tensor-parallel over hidden units (128 per core, all 4 gates),
    with a tiny AllGather of h after each layer. Matmuls are batch-major
    [32 x 512] so the TensorEngine runs fp32r at full rate; biases are
    injected as K=1 ones-row matmuls.
  - Output layer is tensor-parallel over vocab (Wout pre-transposed,
    padded to 51200, 6400 columns per core).
  - The embedding table never touches the device: the two row-gathers
    happen on the host.

Weights are pre-transposed on the host so every big DMA is contiguous
along the partition line. A dummy AllGather issued at t=0 absorbs the
collective channel's one-time setup latency off the critical path.
"""

import os

import numpy as np

B, S, H, E, V, L = 32, 2048, 1024, 1024, 50257, 4
NC = 8
BPC = B // NC          # batches per core (attention)
UPC = H // NC          # hidden units per core (LSTM)
VPAD = 51200           # vocab padded to 8*6400
VS = VPAD // NC        # vocab shard per core
ST = S // 128          # s-tiles per batch (16)
LGC = 512              # logits column chunk

_COMPILED = {}
LAST_EXEC_TIME_NS = None


def _build_nc():
    import concourse.bacc as bacc
    import concourse.mybir as mybir
    import concourse.tile as tile

    f32 = mybir.dt.float32
    f32r = mybir.dt.float32r
    AF = mybir.ActivationFunctionType
    OP = mybir.AluOpType

    nc = bacc.Bacc("TRN2", target_bir_lowering=False, debug=False, num_devices=NC)

    # ---- parameters (f32r is bit-identical to f32 host-side) ----
    encs = nc.declare_dram_parameter("encs", [BPC, S, H], f32r, isOutput=False)
    xT = nc.declare_dram_parameter("xT", [2 * E, B], f32r, isOutput=False)
    h0T = nc.declare_dram_parameter("h0T", [L, H, B], f32r, isOutput=False)
    c0s = nc.declare_dram_parameter("c0s", [L, B, UPC], f32, isOutput=False)
    u_p = nc.declare_dram_parameter("u", [1, H], f32, isOutput=False)
    wih0T = nc.declare_dram_parameter("wih0T", [2 * E, 4 * UPC], f32r, isOutput=False)
    whh0T = nc.declare_dram_parameter("whh0T", [H, 4 * UPC], f32r, isOutput=False)
    wihrT = nc.declare_dram_parameter("wihrT", [L - 1, H, 4 * UPC], f32r, isOutput=False)
    whhrT = nc.declare_dram_parameter("whhrT", [L - 1, H, 4 * UPC], f32r, isOutput=False)
    b0r = nc.declare_dram_parameter("b0r", [1, 4 * UPC], f32r, isOutput=False)
    brr = nc.declare_dram_parameter("brr", [1, (L - 1) * 4 * UPC], f32r, isOutput=False)
    walT = nc.declare_dram_parameter("walT", [2 * H + E, UPC], f32r, isOutput=False)
    balr = nc.declare_dram_parameter("balr", [1, UPC], f32r, isOutput=False)
    woutT = nc.declare_dram_parameter("woutT", [H, VS], f32r, isOutput=False)
    bout_p = nc.declare_dram_parameter("bout", [1, VS], f32r, isOutput=False)

    logits_o = nc.declare_dram_parameter("logits_o", [B, VS], f32, isOutput=True)
    hn_o = nc.declare_dram_parameter("hn_o", [L, B, UPC], f32r, isOutput=True)
    cn_o = nc.declare_dram_parameter("cn_o", [L, B, UPC], f32, isOutput=True)
    attnsT_o = nc.declare_dram_parameter("attnsT", [BPC, S], f32, isOutput=True)

    grp = [list(range(NC))]

    with tile.TileContext(nc) as tc:
        with (
            tc.tile_pool(name="sb", bufs=1) as sb,
            tc.tile_pool(name="ps", bufs=1, space="PSUM") as ps,
            tc.tile_pool(name="dram", bufs=1, space="DRAM") as dram,
        ):
            # ---------- collective channel warmup (absorbs one-time setup) ----
            warm_in = dram.tile([1, 4], f32, name="warm_in")
            warm_out = dram.tile([NC, 4], f32, name="warm_out")
            warm_back = dram.tile([1, 4], f32, name="warm_back")
            wz = sb.tile([1, 4], f32, name="wz")
            nc.vector.memset(wz[:], 0.0)
            nc.gpsimd.dma_start(warm_in[:], wz[:])
            nc.gpsimd.collective_compute(
                "AllGather", OP.bypass, replica_groups=grp,
                ins=[warm_in.opt()], outs=[warm_out.opt()],
            )
            nc.gpsimd.dma_start(warm_back[:], warm_out[0:1, :])

            # ---------- prelude: small constants ----------
            ones = sb.tile([128, 32], f32, name="ones")
            nc.vector.memset(ones[:], 1.0)
            ones_r = sb.tile([1, 32], f32r, name="ones_r")
            nc.vector.tensor_copy(ones_r[:], ones[0:1, :])
            u_bc = sb.tile([128, H], f32, name="u_bc")
            nc.sync.dma_start(u_bc[:], u_p.ap().broadcast_to([128, H]))
            b0r_sb = sb.tile([1, 4 * UPC], f32r, name="b0r_sb")
            nc.sync.dma_start(b0r_sb[:], b0r[:])
            brr_sb = sb.tile([1, (L - 1) * 4 * UPC], f32r, name="brr_sb")
            nc.sync.dma_start(brr_sb[:], brr[:])
            balr_sb = sb.tile([1, UPC], f32r, name="balr_sb")
            nc.sync.dma_start(balr_sb[:], balr[:])
            c0_sb = []
            for l in range(L):
                c0_l = sb.tile([B, UPC], f32, name=f"c0_{l}")
                nc.sync.dma_start(c0_l[:], c0s[l])
                c0_sb.append(c0_l)
            bout_sb = sb.tile([1, VS], f32r, name="bout_sb")
            nc.sync.dma_start(bout_sb[:], bout_p[:])

            # DRAM bounce buffers for collectives
            ag_in = [dram.tile([UPC, B], f32r, name=f"ag_in{l}") for l in range(L)]
            ag_out = [dram.tile([H, B], f32r, name=f"ag_out{l}") for l in range(L)]
            ctx_loc = dram.tile([BPC, H], f32r, name="ctx_loc")
            ctx_all = dram.tile([B, H], f32r, name="ctx_all")
            hid_in = dram.tile([UPC, B], f32r, name="hid_in")
            hid_out = dram.tile([H, B], f32r, name="hid_out")
            invd_dram = dram.tile([1, BPC], f32, name="invd_dram")

            # ---------- LSTM: 4 layers, batch-major fp32r matmuls ----------
            for l in range(L):
                if l == 0:
                    wsrc = [(wih0T, j) for j in range((2 * E) // 128)] + [
                        (whh0T, j) for j in range(H // 128)
                    ]
                    lsrc = [(xT, j) for j in range((2 * E) // 128)] + [
                        (h0T, (0, j)) for j in range(H // 128)
                    ]
                else:
                    wsrc = [(wihrT, (l - 1, j)) for j in range(H // 128)] + [
                        (whhrT, (l - 1, j)) for j in range(H // 128)
                    ]
                    lsrc = [(ag_out[l - 1], j) for j in range(H // 128)] + [
                        (h0T, (l, j)) for j in range(H // 128)
                    ]
                kt = len(wsrc)

                def _dma_src(eng, dst, src, idx):
                    if isinstance(idx, tuple):
                        li, j = idx
                        eng.dma_start(dst[:], src[li, j * 128:(j + 1) * 128, :])
                    else:
                        eng.dma_start(dst[:], src[idx * 128:(idx + 1) * 128, :])

                gp = ps.tile([B, 4 * UPC], f32, name="gp", tag="gp", bufs=1)
                for k in range(kt):
                    wt = sb.tile([128, 4 * UPC], f32r, name="lw", tag="lw", bufs=8)
                    _dma_src(nc.gpsimd, wt, *wsrc[k])
                    rt = sb.tile([128, B], f32r, name="rk", tag="rk", bufs=26)
                    _dma_src(nc.gpsimd, rt, *lsrc[k])
                    nc.tensor.matmul(
                        gp[:], rt[:], wt[:], start=(k == 0), stop=False,
                    )
                brow = b0r_sb[0:1, :] if l == 0 else brr_sb[0:1, (l - 1) * 4 * UPC:l * 4 * UPC]
                nc.tensor.matmul(gp[:], ones_r[:], brow, start=False, stop=True)

                s_i = sb.tile([B, UPC], f32, name="s_i", tag="s_i", bufs=2)
                s_f = sb.tile([B, UPC], f32, name="s_f", tag="s_f", bufs=2)
                s_o = sb.tile([B, UPC], f32, name="s_o", tag="s_o", bufs=2)
                tg = sb.tile([B, UPC], f32, name="tg", tag="tg", bufs=2)
                nc.scalar.activation(s_i[:], gp[:, 0:UPC], AF.Sigmoid)
                nc.scalar.activation(s_f[:], gp[:, UPC:2 * UPC], AF.Sigmoid)
                nc.scalar.activation(s_o[:], gp[:, 3 * UPC:4 * UPC], AF.Sigmoid)
                nc.scalar.activation(tg[:], gp[:, 2 * UPC:3 * UPC], AF.Tanh)

                c_new = sb.tile([B, UPC], f32, name="c_new", tag="c_new", bufs=2)
                m2 = sb.tile([B, UPC], f32, name="m2", tag="m2", bufs=2)
                tcn = sb.tile([B, UPC], f32, name="tcn", tag="tcn", bufs=2)
                h_new = sb.tile([B, UPC], f32r, name="h_new", tag="h_new", bufs=2)
                nc.vector.tensor_mul(c_new[:], s_f[:], c0_sb[l][:])
                nc.vector.tensor_mul(m2[:], s_i[:], tg[:])
                nc.vector.tensor_add(c_new[:], c_new[:], m2[:])
                nc.scalar.activation(tcn[:], c_new[:], AF.Tanh)
                nc.vector.tensor_mul(h_new[:], s_o[:], tcn[:])

                nc.gpsimd.dma_start(cn_o[l], c_new[:])
                nc.gpsimd.dma_start(hn_o[l], h_new[:])
                nc.gpsimd.dma_start(ag_in[l].rearrange("u b -> b u"), h_new[:])
                nc.gpsimd.collective_compute(
                    "AllGather", OP.bypass, replica_groups=grp,
                    ins=[ag_in[l].opt()], outs=[ag_out[l].opt()],
                )

            # ---------- attention: stream enc once; softmax+context in PSUM ----
            e_all, p_all = [], []
            for b in range(BPC):
                e_all.append(sb.tile([128, ST], f32, name=f"e_all{b}"))
                p_all.append(sb.tile([128, ST], f32r, name=f"p_all{b}"))
            scratch = sb.tile([128, H], f32, name="scratch")
            d_ps = ps.tile([1, BPC], f32, name="d_ps")

            for b in range(BPC):
                cpa = ps.tile([1, 512], f32, name="cpa", tag="cpa", bufs=1)
                cpb = ps.tile([1, 512], f32, name="cpb", tag="cpb", bufs=1)
                for t in range(ST):
                    et = sb.tile([128, H], f32r, name="enc_t", tag="enc_t", bufs=6)
                    nc.sync.dma_start(et[:], encs[b, t * 128:(t + 1) * 128, :])
                    nc.vector.scalar_tensor_tensor(
                        out=scratch[:], in0=et.bitcast(f32)[:], scalar=1.0,
                        in1=u_bc[:], op0=OP.mult, op1=OP.mult,
                        accum_out=e_all[b][:, t:t + 1],
                    )
                    nc.scalar.activation(
                        p_all[b][:, t:t + 1], e_all[b][:, t:t + 1], AF.Exp
                    )
                    nc.tensor.matmul(
                        cpa[:], p_all[b][:, t:t + 1], et[:, 0:512],
                        start=(t == 0), stop=(t == ST - 1),
                    )
                    nc.tensor.matmul(
                        cpb[:], p_all[b][:, t:t + 1], et[:, 512:1024],
                        start=(t == 0), stop=(t == ST - 1),
                    )
                d_col = sb.tile([128, 1], f32, name="d_col", tag="d_col", bufs=2)
                nc.vector.reduce_sum(
                    d_col[:], p_all[b].bitcast(f32)[:], axis=mybir.AxisListType.X
                )
                nc.tensor.matmul(
                    d_ps[0:1, b:b + 1], d_col[:], ones[:, 0:1], start=True, stop=True
                )
                db = sb.tile([1, 1], f32, name="db", tag="db", bufs=2)
                invb = sb.tile([1, 1], f32, name="invb", tag="invb", bufs=2)
                nc.vector.tensor_copy(db[:], d_ps[0:1, b:b + 1])
                nc.vector.reciprocal(invb[:], db[:])
                cx_sb = sb.tile([1, H], f32r, name="cx_sb", tag="cx_sb", bufs=2)
                nc.vector.tensor_scalar_mul(cx_sb[:, 0:512], cpa[:], invb[0:1, 0:1])
                nc.vector.tensor_scalar_mul(cx_sb[:, 512:1024], cpb[:], invb[0:1, 0:1])
                nc.gpsimd.dma_start(ctx_loc[b:b + 1, :], cx_sb[:])

            d_sb = sb.tile([1, BPC], f32, name="d_sb")
            inv_d = sb.tile([1, BPC], f32, name="inv_d")
            nc.vector.tensor_copy(d_sb[:], d_ps[:])
            nc.vector.reciprocal(inv_d[:], d_sb[:])
            nc.gpsimd.dma_start(invd_dram[:], inv_d[:])
            invd_bc = sb.tile([128, BPC], f32, name="invd_bc")
            nc.gpsimd.dma_start(invd_bc[:], invd_dram.broadcast_to([128, BPC]))

            nc.gpsimd.collective_compute(
                "AllGather", OP.bypass, replica_groups=grp,
                ins=[ctx_loc.opt()], outs=[ctx_all.opt()],
            )

            for b in range(BPC):
                at_sb = sb.tile([128, ST], f32, name="at_sb", tag="at_sb", bufs=2)
                nc.vector.tensor_scalar_mul(
                    at_sb[:], p_all[b].bitcast(f32)[:], invd_bc[:, b:b + 1]
                )
                nc.gpsimd.dma_start(
                    attnsT_o[b].rearrange("(t p) -> p t", p=128), at_sb[:]
                )

            # ---------- hidden = tanh(Wal @ [out; context; key_e] + bal) -------
            KT = (2 * H + E) // 128  # 24
            wal_tiles = []
            for k in range(KT):
                wt = sb.tile([128, UPC], f32r, name="walt", tag="walt", bufs=KT)
                nc.sync.dma_start(wt[:], walT[k * 128:(k + 1) * 128, :])
                wal_tiles.append(wt)
            hr_tiles = []
            for k in range(KT):
                rt = sb.tile([128, B], f32r, name="hrk", tag="rk", bufs=26)
                if k < 8:
                    nc.gpsimd.dma_start(rt[:], ag_out[L - 1][k * 128:(k + 1) * 128, :])
                elif k < 16:
                    j = k - 8
                    nc.gpsimd.dma_start(
                        rt[:],
                        ctx_all[:, j * 128:(j + 1) * 128].rearrange("b h -> h b"),
                    )
                else:
                    j = k - 16
                    nc.gpsimd.dma_start(rt[:], xT[E + j * 128:E + (j + 1) * 128, :])
                hr_tiles.append(rt)

            hid_ps = ps.tile([B, UPC], f32, name="hid_ps")
            for k in range(KT):
                nc.tensor.matmul(
                    hid_ps[:], hr_tiles[k][:], wal_tiles[k][:],
                    start=(k == 0), stop=False,
                )
            nc.tensor.matmul(hid_ps[:], ones_r[:], balr_sb[:], start=False, stop=True)
            hid_sb = sb.tile([B, UPC], f32r, name="hid_sb")
            nc.scalar.activation(hid_sb[:], hid_ps[:], AF.Tanh)
            nc.gpsimd.dma_start(hid_in.rearrange("u b -> b u"), hid_sb[:])
            nc.gpsimd.collective_compute(
                "AllGather", OP.bypass, replica_groups=grp,
                ins=[hid_in.opt()], outs=[hid_out.opt()],
            )
            hT_sb = sb.tile([128, 8 * B], f32r, name="hT_sb")
            for k in range(8):
                nc.gpsimd.dma_start(
                    hT_sb[:, k * B:(k + 1) * B], hid_out[k * 128:(k + 1) * 128, :]
                )

            # ---------- logits: vocab-sharded output projection (fp32r) -------
            n_ch = (VS + LGC - 1) // LGC
            for ch in range(n_ch):
                c0_ = ch * LGC
                cw = min(LGC, VS - c0_)
                lg = ps.tile([B, LGC], f32, name="lg", tag="lg", bufs=2)
                for k in range(8):
                    wt = sb.tile([128, LGC], f32r, name="wo", tag="wo", bufs=34)
                    nc.sync.dma_start(
                        wt[:, 0:cw], woutT[k * 128:(k + 1) * 128, c0_:c0_ + cw]
                    )
                    nc.tensor.matmul(
                        lg[:, 0:cw], hT_sb[:, k * B:(k + 1) * B], wt[:, 0:cw],
                        start=(k == 0), stop=False,
                    )
                nc.tensor.matmul(
                    lg[:, 0:cw], ones_r[:], bout_sb[0:1, c0_:c0_ + cw],
                    start=False, stop=True,
                )
                lsb = sb.tile([B, LGC], f32, name="lsb", tag="lsb", bufs=3)
                nc.vector.tensor_copy(lsb[:, 0:cw], lg[:, 0:cw])
                nc.sync.dma_start(logits_o[:, c0_:c0_ + cw], lsb[:, 0:cw])

    nc.finalize()
    return nc


def _host_prep(encoder_outputs, input_seq, keyword, h0, c0, emb,
               Wih0, Whh0, bih0, bhh0, Wih_r, Whh_r, bih_r, bhh_r,
               Wa, ba, v, Wal, bal, Wout, bout):
    f = np.float32
    enc = np.asarray(encoder_outputs, f)
    emb = np.asarray(emb, f)
    idx_sos = np.asarray(input_seq).astype(np.int64)[:, 0]
    idx_key = np.asarray(keyword).astype(np.int64)
    sos = emb[idx_sos]                      # [B, E]
    key_e = emb[idx_key]                    # [B, E]
    xT = np.ascontiguousarray(np.concatenate([sos, key_e], axis=1).T)  # [2E, B]

    Wa = np.asarray(Wa, f)
    v_ = np.asarray(v, f)
    u = (Wa[:, H:].T @ v_)[None, :].astype(f)  # [1, H]

    h0 = np.asarray(h0, f)
    c0 = np.asarray(c0, f)
    h0T = np.ascontiguousarray(h0.transpose(0, 2, 1))  # [L, H, B]

    Wih0 = np.asarray(Wih0, f).reshape(4, H, 2 * E)
    Whh0 = np.asarray(Whh0, f).reshape(4, H, H)
    Wih_r = np.asarray(Wih_r, f).reshape(L - 1, 4, H, H)
    Whh_r = np.asarray(Whh_r, f).reshape(L - 1, 4, H, H)
    b0_full = (np.asarray(bih0, f) + np.asarray(bhh0, f)).reshape(4, H)
    br_full = (np.asarray(bih_r, f) + np.asarray(bhh_r, f)).reshape(L - 1, 4, H)

    Wal = np.asarray(Wal, f)
    bal = np.asarray(bal, f)
    Wout = np.asarray(Wout, f)
    bout = np.asarray(bout, f)

    woutT_pad = np.zeros((H, VPAD), f)
    woutT_pad[:, :V] = Wout.T
    bout_pad = np.zeros(VPAD, f)
    bout_pad[:V] = bout

    in_maps = []
    for c in range(NC):
        sl = slice(c * UPC, (c + 1) * UPC)
        wih0T = np.ascontiguousarray(
            Wih0[:, sl, :].reshape(4 * UPC, 2 * E).T)        # [2E, 512]
        whh0T = np.ascontiguousarray(
            Whh0[:, sl, :].reshape(4 * UPC, H).T)            # [H, 512]
        wihrT = np.ascontiguousarray(
            Wih_r[:, :, sl, :].reshape(L - 1, 4 * UPC, H).transpose(0, 2, 1))
        whhrT = np.ascontiguousarray(
            Whh_r[:, :, sl, :].reshape(L - 1, 4 * UPC, H).transpose(0, 2, 1))
        in_maps.append({
            "encs": np.ascontiguousarray(enc[c * BPC:(c + 1) * BPC]),
            "xT": xT,
            "h0T": h0T,
            "c0s": np.ascontiguousarray(c0[:, :, sl]),               # [L, B, 128]
            "u": u,
            "wih0T": wih0T,
            "whh0T": whh0T,
            "wihrT": wihrT,
            "whhrT": whhrT,
            "b0r": np.ascontiguousarray(b0_full[:, sl].reshape(1, 4 * UPC)),
            "brr": np.ascontiguousarray(br_full[:, :, sl].reshape(1, (L - 1) * 4 * UPC)),
            "walT": np.ascontiguousarray(Wal[sl, :].T),              # [3072, 128]
            "balr": np.ascontiguousarray(bal[sl])[None, :],          # [1, 128]
            "woutT": np.ascontiguousarray(woutT_pad[:, c * VS:(c + 1) * VS]),
            "bout": np.ascontiguousarray(bout_pad[c * VS:(c + 1) * VS])[None, :],
        })
    return in_maps, key_e


def kernel(**inputs):
    global LAST_EXEC_TIME_NS
    from concourse.bass_utils import run_bass_kernel_spmd

    if "nc" not in _COMPILED:
        _COMPILED["nc"] = _build_nc()
    nc = _COMPILED["nc"]

    in_maps, _ = _host_prep(**inputs)

    trace = os.environ.get("KERNEL_TRACE", "0") == "1"
    res = run_bass_kernel_spmd(nc, in_maps, list(range(NC)), trace=trace)
    LAST_EXEC_TIME_NS = res.exec_time_ns
    _COMPILED["last_res"] = res

    r = res.results
    logits = np.concatenate([r[c]["logits_o"] for c in range(NC)], axis=1)[:, :V]
    hn = np.concatenate([r[c]["hn_o"] for c in range(NC)], axis=2)
    cn = np.concatenate([r[c]["cn_o"] for c in range(NC)], axis=2)
    attns = np.concatenate([r[c]["attnsT"] for c in range(NC)], axis=0)[:, None, :]
    return logits, hn, cn, attns


# revision 12
# speedup vs baseline: 1.0267x; 1.0267x over previous
"""Trainium2 Bass kernel for a single-step 4-layer LSTM decoder with
Bahdanau attention and a 50k-vocab output projection, SPMD across 8
NeuronCores.

Distribution (hardcoded for B=32, S=2048, H=E=1024, V=50257, L=4):
  - Attention is data-parallel over batch (4 per core). The additive
    energy's hidden term is constant per row and cancels in softmax, and v
    folds through Wa_e on the host into one vector u, so the whole
    attention scan is independent of the LSTM and streams encoder_outputs
    exactly once (unnormalized softmax accumulated in PSUM via rank-1
    fp32r matmuls).
  - LSTM is tensor-parallel over hidden units (128 per core, all 4 gates),
    with a tiny AllGather of h after each layer. Matmuls are batch-major
    [32 x 512] so the TensorEngine runs fp32r at full rate; biases are
    injected as K=1 ones-row matmuls.
  - Output layer is tensor-parallel over vocab (Wout pre-transposed,
    padded to 51200, 6400 columns per core).
  - The embedding table never touches the device: the two row-gathers
    happen on the host.

Weights are pre-transposed on the host so every big DMA is contiguous
along the partition line. A dummy AllGather issued at t=0 absorbs the
collective channel's one-time setup latency off the critical path.
"""

import os

import numpy as np

B, S, H, E, V, L = 32, 2048, 1024, 1024, 50257, 4
NC = 8
BPC = B // NC          # batches per core (attention)
UPC = H // NC          # hidden units per core (LSTM)
VPAD = 51200           # vocab padded to 8*6400
VS = VPAD // NC        # vocab shard per core
ST = S // 128          # s-tiles per batch (16)
LGC = 512              # logits column chunk

_COMPILED = {}
LAST_EXEC_TIME_NS = None


def _build_nc():
    import concourse.bacc as bacc
    import concourse.mybir as mybir
    import concourse.tile as tile

    f32 = mybir.dt.float32
    f32r = mybir.dt.float32r
    AF = mybir.ActivationFunctionType
    OP = mybir.AluOpType

    nc = bacc.Bacc("TRN2", target_bir_lowering=False, debug=False, num_devices=NC)

    # ---- parameters (f32r is bit-identical to f32 host-side) ----
    encs = nc.declare_dram_parameter("encs", [BPC, S, H], f32r, isOutput=False)
    xT = nc.declare_dram_parameter("xT", [2 * E, B], f32r, isOutput=False)
    h0T = nc.declare_dram_parameter("h0T", [L, H, B], f32r, isOutput=False)
    c0s = nc.declare_dram_parameter("c0s", [L, B, UPC], f32, isOutput=False)
    u_p = nc.declare_dram_parameter("u", [1, H], f32, isOutput=False)
    wih0T = nc.declare_dram_parameter("wih0T", [2 * E, 4 * UPC], f32r, isOutput=False)
    whh0T = nc.declare_dram_parameter("whh0T", [H, 4 * UPC], f32r, isOutput=False)
    wihrT = nc.declare_dram_parameter("wihrT", [L - 1, H, 4 * UPC], f32r, isOutput=False)
    whhrT = nc.declare_dram_parameter("whhrT", [L - 1, H, 4 * UPC], f32r, isOutput=False)
    b0r = nc.declare_dram_parameter("b0r", [1, 4 * UPC], f32r, isOutput=False)
    brr = nc.declare_dram_parameter("brr", [1, (L - 1) * 4 * UPC], f32r, isOutput=False)
    walT = nc.declare_dram_parameter("walT", [2 * H + E, UPC], f32r, isOutput=False)
    balr = nc.declare_dram_parameter("balr", [1, UPC], f32r, isOutput=False)
    woutT = nc.declare_dram_parameter("woutT", [H, VS], f32r, isOutput=False)
    bout_p = nc.declare_dram_parameter("bout", [1, VS], f32r, isOutput=False)

    logits_o = nc.declare_dram_parameter("logits_o", [B, VS], f32, isOutput=True)
    hn_o = nc.declare_dram_parameter("hn_o", [L, B, UPC], f32r, isOutput=True)
    cn_o = nc.declare_dram_parameter("cn_o", [L, B, UPC], f32, isOutput=True)
    attnsT_o = nc.declare_dram_parameter("attnsT", [BPC, S], f32, isOutput=True)

    grp = [list(range(NC))]

    with tile.TileContext(nc) as tc:
        with (
            tc.tile_pool(name="sb", bufs=1) as sb,
            tc.tile_pool(name="ps", bufs=1, space="PSUM") as ps,
            tc.tile_pool(name="dram", bufs=1, space="DRAM") as dram,
        ):
            # ---------- collective channel warmup (absorbs one-time setup) ----
            warm_in = dram.tile([1, 4], f32, name="warm_in")
            warm_out = dram.tile([NC, 4], f32, name="warm_out")
            warm_back = dram.tile([1, 4], f32, name="warm_back")
            wz = sb.tile([1, 4], f32, name="wz")
            nc.vector.memset(wz[:], 0.0)
            nc.gpsimd.dma_start(warm_in[:], wz[:])
            nc.gpsimd.collective_compute(
                "AllGather", OP.bypass, replica_groups=grp,
                ins=[warm_in.opt()], outs=[warm_out.opt()],
            )
            nc.gpsimd.dma_start(warm_back[:], warm_out[0:1, :])

            # ---------- prelude: small constants ----------
            ones = sb.tile([128, 32], f32, name="ones")
            nc.vector.memset(ones[:], 1.0)
            ones_r = sb.tile([1, 32], f32r, name="ones_r")
            nc.vector.tensor_copy(ones_r[:], ones[0:1, :])
            u_bc = sb.tile([128, H], f32, name="u_bc")
            nc.sync.dma_start(u_bc[:], u_p.ap().broadcast_to([128, H]))
            b0r_sb = sb.tile([1, 4 * UPC], f32r, name="b0r_sb")
            nc.sync.dma_start(b0r_sb[:], b0r[:])
            brr_sb = sb.tile([1, (L - 1) * 4 * UPC], f32r, name="brr_sb")
            nc.sync.dma_start(brr_sb[:], brr[:])
            balr_sb = sb.tile([1, UPC], f32r, name="balr_sb")
            nc.sync.dma_start(balr_sb[:], balr[:])
            c0_sb = []
            for l in range(L):
                c0_l = sb.tile([B, UPC], f32, name=f"c0_{l}")
                nc.sync.dma_start(c0_l[:], c0s[l])
                c0_sb.append(c0_l)

            # DRAM bounce buffers for collectives
            ag_in = [dram.tile([UPC, B], f32r, name=f"ag_in{l}") for l in range(L)]
            ag_out = [dram.tile([H, B], f32r, name=f"ag_out{l}") for l in range(L)]
            ctx_loc = dram.tile([BPC, H], f32r, name="ctx_loc")
            ctx_all = dram.tile([B, H], f32r, name="ctx_all")
            hid_in = dram.tile([UPC, B], f32r, name="hid_in")
            hid_out = dram.tile([H, B], f32r, name="hid_out")
            invd_dram = dram.tile([1, BPC], f32, name="invd_dram")

            # ----- LSTM layers software-pipelined with attention batches -----
            e_all, p_all = [], []
            for b in range(BPC):
                e_all.append(sb.tile([128, ST], f32, name=f"e_all{b}"))
                p_all.append(sb.tile([128, ST], f32r, name=f"p_all{b}"))
            scratch = sb.tile([128, H], f32, name="scratch")
            d_ps = ps.tile([1, BPC], f32, name="d_ps")

            def emit_lstm_layer(l):
                if l == 0:
                    wsrc = [(wih0T, j) for j in range((2 * E) // 128)] + [
                        (whh0T, j) for j in range(H // 128)
                    ]
                    lsrc = [(xT, j) for j in range((2 * E) // 128)] + [
                        (h0T, (0, j)) for j in range(H // 128)
                    ]
                else:
                    wsrc = [(wihrT, (l - 1, j)) for j in range(H // 128)] + [
                        (whhrT, (l - 1, j)) for j in range(H // 128)
                    ]
                    lsrc = [(ag_out[l - 1], j) for j in range(H // 128)] + [
                        (h0T, (l, j)) for j in range(H // 128)
                    ]
                kt = len(wsrc)

                def _dma_src(eng, dst, src_, idx):
                    if isinstance(idx, tuple):
                        li, j = idx
                        eng.dma_start(dst[:], src_[li, j * 128:(j + 1) * 128, :])
                    else:
                        eng.dma_start(dst[:], src_[idx * 128:(idx + 1) * 128, :])

                gp = ps.tile([B, 4 * UPC], f32, name="gp", tag="gp", bufs=1)
                for k in range(kt):
                    wt = sb.tile([128, 4 * UPC], f32r, name="lw", tag="lw", bufs=24)
                    _dma_src(nc.scalar, wt, *wsrc[k])
                    rt = sb.tile([128, B], f32r, name="rk", tag="rk", bufs=26)
                    _dma_src(nc.gpsimd, rt, *lsrc[k])
                    nc.tensor.matmul(
                        gp[:], rt[:], wt[:], start=(k == 0), stop=False,
                    )
                brow = (b0r_sb[0:1, :] if l == 0
                        else brr_sb[0:1, (l - 1) * 4 * UPC:l * 4 * UPC])
                nc.tensor.matmul(gp[:], ones_r[:], brow, start=False, stop=True)

                s_i = sb.tile([B, UPC], f32, name="s_i", tag="s_i", bufs=2)
                s_f = sb.tile([B, UPC], f32, name="s_f", tag="s_f", bufs=2)
                s_o = sb.tile([B, UPC], f32, name="s_o", tag="s_o", bufs=2)
                tg = sb.tile([B, UPC], f32, name="tg", tag="tg", bufs=2)
                nc.scalar.activation(s_i[:], gp[:, 0:UPC], AF.Sigmoid)
                nc.scalar.activation(s_f[:], gp[:, UPC:2 * UPC], AF.Sigmoid)
                nc.scalar.activation(s_o[:], gp[:, 3 * UPC:4 * UPC], AF.Sigmoid)
                nc.scalar.activation(tg[:], gp[:, 2 * UPC:3 * UPC], AF.Tanh)

                c_new = sb.tile([B, UPC], f32, name="c_new", tag="c_new", bufs=2)
                m2 = sb.tile([B, UPC], f32, name="m2", tag="m2", bufs=2)
                tcn = sb.tile([B, UPC], f32, name="tcn", tag="tcn", bufs=2)
                h_new = sb.tile([B, UPC], f32r, name="h_new", tag="h_new", bufs=2)
                nc.vector.tensor_mul(c_new[:], s_f[:], c0_sb[l][:])
                nc.vector.tensor_mul(m2[:], s_i[:], tg[:])
                nc.vector.tensor_add(c_new[:], c_new[:], m2[:])
                nc.scalar.activation(tcn[:], c_new[:], AF.Tanh)
                nc.vector.tensor_mul(h_new[:], s_o[:], tcn[:])

                nc.gpsimd.dma_start(cn_o[l], c_new[:])
                nc.gpsimd.dma_start(hn_o[l], h_new[:])
                nc.gpsimd.dma_start(ag_in[l].rearrange("u b -> b u"), h_new[:])
                nc.gpsimd.collective_compute(
                    "AllGather", OP.bypass, replica_groups=grp,
                    ins=[ag_in[l].opt()], outs=[ag_out[l].opt()],
                )

            def emit_attn_batch(b):
                cpa = ps.tile([1, 512], f32, name="cpa", tag="cpa", bufs=1)
                cpb = ps.tile([1, 512], f32, name="cpb", tag="cpb", bufs=1)
                for t in range(ST):
                    et = sb.tile([128, H], f32r, name="enc_t", tag="enc_t", bufs=6)
                    nc.sync.dma_start(et[:], encs[b, t * 128:(t + 1) * 128, :])
                    nc.vector.scalar_tensor_tensor(
                        out=scratch[:], in0=et.bitcast(f32)[:], scalar=1.0,
                        in1=u_bc[:], op0=OP.mult, op1=OP.mult,
                        accum_out=e_all[b][:, t:t + 1],
                    )
                    nc.scalar.activation(
                        p_all[b][:, t:t + 1], e_all[b][:, t:t + 1], AF.Exp
                    )
                    nc.tensor.matmul(
                        cpa[:], p_all[b][:, t:t + 1], et[:, 0:512],
                        start=(t == 0), stop=(t == ST - 1),
                    )
                    nc.tensor.matmul(
                        cpb[:], p_all[b][:, t:t + 1], et[:, 512:1024],
                        start=(t == 0), stop=(t == ST - 1),
                    )
                d_col = sb.tile([128, 1], f32, name="d_col", tag="d_col", bufs=2)
                nc.vector.reduce_sum(
                    d_col[:], p_all[b].bitcast(f32)[:], axis=mybir.AxisListType.X
                )
                nc.tensor.matmul(
                    d_ps[0:1, b:b + 1], d_col[:], ones[:, 0:1], start=True, stop=True
                )
                db = sb.tile([1, 1], f32, name="db", tag="db", bufs=2)
                invb = sb.tile([1, 1], f32, name="invb", tag="invb", bufs=2)
                nc.vector.tensor_copy(db[:], d_ps[0:1, b:b + 1])
                nc.vector.reciprocal(invb[:], db[:])
                cx_sb = sb.tile([1, H], f32r, name="cx_sb", tag="cx_sb", bufs=2)
                nc.vector.tensor_scalar_mul(cx_sb[:, 0:512], cpa[:], invb[0:1, 0:1])
                nc.vector.tensor_scalar_mul(cx_sb[:, 512:1024], cpb[:], invb[0:1, 0:1])
                nc.gpsimd.dma_start(ctx_loc[b:b + 1, :], cx_sb[:])

            for i in range(4):
                emit_lstm_layer(i)
                emit_attn_batch(i)

            nc.gpsimd.collective_compute(
                "AllGather", OP.bypass, replica_groups=grp,
                ins=[ctx_loc.opt()], outs=[ctx_all.opt()],
            )

            # ---------- hidden = tanh(Wal @ [out; context; key_e] + bal) -------
            KT = (2 * H + E) // 128  # 24
            wal_tiles = []
            for k in range(KT):
                wt = sb.tile([128, UPC], f32r, name="walt", tag="walt", bufs=KT)
                nc.sync.dma_start(wt[:], walT[k * 128:(k + 1) * 128, :])
                wal_tiles.append(wt)
            hr_tiles = []
            for k in range(KT):
                rt = sb.tile([128, B], f32r, name="hrk", tag="rk", bufs=26)
                if k < 8:
                    nc.gpsimd.dma_start(rt[:], ag_out[L - 1][k * 128:(k + 1) * 128, :])
                elif k < 16:
                    j = k - 8
                    nc.gpsimd.dma_start(
                        rt[:],
                        ctx_all[:, j * 128:(j + 1) * 128].rearrange("b h -> h b"),
                    )
                else:
                    j = k - 16
                    nc.gpsimd.dma_start(rt[:], xT[E + j * 128:E + (j + 1) * 128, :])
                hr_tiles.append(rt)

            hid_ps = ps.tile([B, UPC], f32, name="hid_ps")
            for k in range(KT):
                nc.tensor.matmul(
                    hid_ps[:], hr_tiles[k][:], wal_tiles[k][:],
                    start=(k == 0), stop=False,
                )
            nc.tensor.matmul(hid_ps[:], ones_r[:], balr_sb[:], start=False, stop=True)
            hid_sb = sb.tile([B, UPC], f32r, name="hid_sb")
            nc.scalar.activation(hid_sb[:], hid_ps[:], AF.Tanh)
            nc.gpsimd.dma_start(hid_in.rearrange("u b -> b u"), hid_sb[:])
            nc.gpsimd.collective_compute(
                "AllGather", OP.bypass, replica_groups=grp,
                ins=[hid_in.opt()], outs=[hid_out.opt()],
            )
            hT_sb = sb.tile([128, 8 * B], f32r, name="hT_sb")
            for k in range(8):
                nc.gpsimd.dma_start(
                    hT_sb[:, k * B:(k + 1) * B], hid_out[k * 128:(k + 1) * 128, :]
                )

            # ---------- logits: vocab-sharded output projection (fp32r) -------
            n_ch = (VS + LGC - 1) // LGC
            for ch in range(n_ch):
                c0_ = ch * LGC
                cw = min(LGC, VS - c0_)
                lg = ps.tile([B, LGC], f32, name="lg", tag="lg", bufs=2)
                for k in range(8):
                    wt = sb.tile([128, LGC], f32r, name="wo", tag="wo", bufs=34)
                    nc.sync.dma_start(
                        wt[:, 0:cw], woutT[k * 128:(k + 1) * 128, c0_:c0_ + cw]
                    )
                    nc.tensor.matmul(
                        lg[:, 0:cw], hT_sb[:, k * B:(k + 1) * B], wt[:, 0:cw],
                        start=(k == 0), stop=False,
                    )
                bout_sb = sb.tile([1, LGC], f32r, name="bout_sb", tag="bout_sb", bufs=2)
                nc.sync.dma_start(bout_sb[0:1, 0:cw], bout_p[0:1, c0_:c0_ + cw])
                nc.tensor.matmul(
                    lg[:, 0:cw], ones_r[:], bout_sb[0:1, 0:cw],
                    start=False, stop=True,
                )
                lsb = sb.tile([B, LGC], f32, name="lsb", tag="lsb", bufs=3)
                nc.vector.tensor_copy(lsb[:, 0:cw], lg[:, 0:cw])
                nc.sync.dma_start(logits_o[:, c0_:c0_ + cw], lsb[:, 0:cw])


            # ---------- attns outputs (off the critical path) ----------
            d_sb = sb.tile([1, BPC], f32, name="d_sb")
            inv_d = sb.tile([1, BPC], f32, name="inv_d")
            nc.vector.tensor_copy(d_sb[:], d_ps[:])
            nc.vector.reciprocal(inv_d[:], d_sb[:])
            nc.gpsimd.dma_start(invd_dram[:], inv_d[:])
            invd_bc = sb.tile([128, BPC], f32, name="invd_bc")
            nc.gpsimd.dma_start(invd_bc[:], invd_dram.broadcast_to([128, BPC]))
            for b in range(BPC):
                at_sb = sb.tile([128, ST], f32, name="at_sb", tag="at_sb", bufs=2)
                nc.vector.tensor_scalar_mul(
                    at_sb[:], p_all[b].bitcast(f32)[:], invd_bc[:, b:b + 1]
                )
                nc.gpsimd.dma_start(
                    attnsT_o[b].rearrange("(t p) -> p t", p=128), at_sb[:]
                )

    nc.finalize()
    return nc


def _host_prep(encoder_outputs, input_seq, keyword, h0, c0, emb,
               Wih0, Whh0, bih0, bhh0, Wih_r, Whh_r, bih_r, bhh_r,
               Wa, ba, v, Wal, bal, Wout, bout):
    f = np.float32
    enc = np.asarray(encoder_outputs, f)
    emb = np.asarray(emb, f)
    idx_sos = np.asarray(input_seq).astype(np.int64)[:, 0]
    idx_key = np.asarray(keyword).astype(np.int64)
    sos = emb[idx_sos]                      # [B, E]
    key_e = emb[idx_key]                    # [B, E]
    xT = np.ascontiguousarray(np.concatenate([sos, key_e], axis=1).T)  # [2E, B]

    Wa = np.asarray(Wa, f)
    v_ = np.asarray(v, f)
    u = (Wa[:, H:].T @ v_)[None, :].astype(f)  # [1, H]

    h0 = np.asarray(h0, f)
    c0 = np.asarray(c0, f)
    h0T = np.ascontiguousarray(h0.transpose(0, 2, 1))  # [L, H, B]

    Wih0 = np.asarray(Wih0, f).reshape(4, H, 2 * E)
    Whh0 = np.asarray(Whh0, f).reshape(4, H, H)
    Wih_r = np.asarray(Wih_r, f).reshape(L - 1, 4, H, H)
    Whh_r = np.asarray(Whh_r, f).reshape(L - 1, 4, H, H)
    b0_full = (np.asarray(bih0, f) + np.asarray(bhh0, f)).reshape(4, H)
    br_full = (np.asarray(bih_r, f) + np.asarray(bhh_r, f)).reshape(L - 1, 4, H)

    Wal = np.asarray(Wal, f)
    bal = np.asarray(bal, f)
    Wout = np.asarray(Wout, f)
    bout = np.asarray(bout, f)

    woutT_pad = np.zeros((H, VPAD), f)
    woutT_pad[:, :V] = Wout.T
    bout_pad = np.zeros(VPAD, f)
    bout_pad[:V] = bout

    in_maps = []
    for c in range(NC):
        sl = slice(c * UPC, (c + 1) * UPC)
        wih0T = np.ascontiguousarray(
            Wih0[:, sl, :].reshape(4 * UPC, 2 * E).T)        # [2E, 512]
        whh0T = np.ascontiguousarray(
            Whh0[:, sl, :].reshape(4 * UPC, H).T)            # [H, 512]
        wihrT = np.ascontiguousarray(
            Wih_r[:, :, sl, :].reshape(L - 1, 4 * UPC, H).transpose(0, 2, 1))
        whhrT = np.ascontiguousarray(
            Whh_r[:, :, sl, :].reshape(L - 1, 4 * UPC, H).transpose(0, 2, 1))
        in_maps.append({
            "encs": np.ascontiguousarray(enc[c * BPC:(c + 1) * BPC]),
            "xT": xT,
            "h0T": h0T,
            "c0s": np.ascontiguousarray(c0[:, :, sl]),               # [L, B, 128]
            "u": u,
            "wih0T": wih0T,
            "whh0T": whh0T,
            "wihrT": wihrT,
            "whhrT": whhrT,
            "b0r": np.ascontiguousarray(b0_full[:, sl].reshape(1, 4 * UPC)),
            "brr": np.ascontiguousarray(br_full[:, :, sl].reshape(1, (L - 1) * 4 * UPC)),
            "walT": np.ascontiguousarray(Wal[sl, :].T),              # [3072, 128]
            "balr": np.ascontiguousarray(bal[sl])[None, :],          # [1, 128]
            "woutT": np.ascontiguousarray(woutT_pad[:, c * VS:(c + 1) * VS]),
            "bout": np.ascontiguousarray(bout_pad[c * VS:(c + 1) * VS])[None, :],
        })
    return in_maps, key_e


def kernel(**inputs):
    global LAST_EXEC_TIME_NS
    from concourse.bass_utils import run_bass_kernel_spmd

    if "nc" not in _COMPILED:
        _COMPILED["nc"] = _build_nc()
    nc = _COMPILED["nc"]

    in_maps, _ = _host_prep(**inputs)

    trace = os.environ.get("KERNEL_TRACE", "0") == "1"
    res = run_bass_kernel_spmd(nc, in_maps, list(range(NC)), trace=trace)
    LAST_EXEC_TIME_NS = res.exec_time_ns
    _COMPILED["last_res"] = res

    r = res.results
    logits = np.concatenate([r[c]["logits_o"] for c in range(NC)], axis=1)[:, :V]
    hn = np.concatenate([r[c]["hn_o"] for c in range(NC)], axis=2)
    cn = np.concatenate([r[c]["cn_o"] for c in range(NC)], axis=2)
    attns = np.concatenate([r[c]["attnsT"] for c in range(NC)], axis=0)[:, None, :]
    return logits, hn, cn, attns


# revision 14
# speedup vs baseline: 1.0850x; 1.0568x over previous
"""Trainium2 Bass kernel for a single-step 4-layer LSTM decoder with
Bahdanau attention and a 50k-vocab output projection, SPMD across 8
NeuronCores.

Distribution (hardcoded for B=32, S=2048, H=E=1024, V=50257, L=4):
  - Attention is data-parallel over batch (4 per core). The additive
    energy's hidden term is constant per row and cancels in softmax, and v
    folds through Wa_e on the host into one vector u, so the whole
    attention scan is independent of the LSTM and streams encoder_outputs
    exactly once (unnormalized softmax accumulated in PSUM via rank-1
    fp32r matmuls).
  - LSTM is tensor-parallel over hidden units (128 per core, all 4 gates),
    with a tiny AllGather of h after each layer. Matmuls are batch-major
    [32 x 512] so the TensorEngine runs fp32r at full rate; biases are
    injected as K=1 ones-row matmuls.
  - Output layer is tensor-parallel over vocab (Wout pre-transposed,
    padded to 51200, 6400 columns per core).
  - The embedding table never touches the device: the two row-gathers
    happen on the host.

Weights are pre-transposed on the host so every big DMA is contiguous
along the partition line. A dummy AllGather issued at t=0 absorbs the
collective channel's one-time setup latency off the critical path.
"""

import os

import numpy as np

B, S, H, E, V, L = 32, 2048, 1024, 1024, 50257, 4
NC = 8
BPC = B // NC          # batches per core (attention)
UPC = H // NC          # hidden units per core (LSTM)
VPAD = 51200           # vocab padded to 8*6400
VS = VPAD // NC        # vocab shard per core
ST = S // 128          # s-tiles per batch (16)
LGC = 512              # logits column chunk

_COMPILED = {}
LAST_EXEC_TIME_NS = None


def _build_nc():
    import concourse.bacc as bacc
    import concourse.mybir as mybir
    import concourse.tile as tile

    f32 = mybir.dt.float32
    f32r = mybir.dt.float32r
    AF = mybir.ActivationFunctionType
    OP = mybir.AluOpType

    nc = bacc.Bacc("TRN2", target_bir_lowering=False, debug=False, num_devices=NC)

    # ---- parameters (f32r is bit-identical to f32 host-side) ----
    encs = nc.declare_dram_parameter("encs", [BPC, S, H], f32r, isOutput=False)
    xT = nc.declare_dram_parameter("xT", [2 * E, B], f32r, isOutput=False)
    h0T = nc.declare_dram_parameter("h0T", [L, H, B], f32r, isOutput=False)
    c0s = nc.declare_dram_parameter("c0s", [L, B, UPC], f32, isOutput=False)
    u_p = nc.declare_dram_parameter("u", [1, H], f32, isOutput=False)
    wih0T = nc.declare_dram_parameter("wih0T", [2 * E, 4 * UPC], f32r, isOutput=False)
    whh0T = nc.declare_dram_parameter("whh0T", [H, 4 * UPC], f32r, isOutput=False)
    wihrT = nc.declare_dram_parameter("wihrT", [L - 1, H, 4 * UPC], f32r, isOutput=False)
    whhrT = nc.declare_dram_parameter("whhrT", [L - 1, H, 4 * UPC], f32r, isOutput=False)
    b0r = nc.declare_dram_parameter("b0r", [1, 4 * UPC], f32r, isOutput=False)
    brr = nc.declare_dram_parameter("brr", [1, (L - 1) * 4 * UPC], f32r, isOutput=False)
    walT = nc.declare_dram_parameter("walT", [2 * H + E, UPC], f32r, isOutput=False)
    balr = nc.declare_dram_parameter("balr", [1, UPC], f32r, isOutput=False)
    woutT = nc.declare_dram_parameter("woutT", [H, VS], f32r, isOutput=False)
    bout_p = nc.declare_dram_parameter("bout", [1, VS], f32r, isOutput=False)

    logits_o = nc.declare_dram_parameter("logits_o", [B, VS], f32, isOutput=True)
    hn_o = nc.declare_dram_parameter("hn_o", [L, B, UPC], f32r, isOutput=True)
    cn_o = nc.declare_dram_parameter("cn_o", [L, B, UPC], f32, isOutput=True)
    attnsT_o = nc.declare_dram_parameter("attnsT", [BPC, S], f32, isOutput=True)

    grp = [list(range(NC))]

    with tile.TileContext(nc) as tc:
        with (
            tc.tile_pool(name="sb", bufs=1) as sb,
            tc.tile_pool(name="ps", bufs=1, space="PSUM") as ps,
            tc.tile_pool(name="dram", bufs=1, space="DRAM") as dram,
        ):
            # ---------- collective channel warmup (absorbs one-time setup) ----
            warm_in = dram.tile([1, 4], f32, name="warm_in")
            warm_out = dram.tile([NC, 4], f32, name="warm_out")
            warm_back = dram.tile([1, 4], f32, name="warm_back")
            wz = sb.tile([1, 4], f32, name="wz")
            nc.vector.memset(wz[:], 0.0)
            nc.gpsimd.dma_start(warm_in[:], wz[:])
            nc.gpsimd.collective_compute(
                "AllGather", OP.bypass, replica_groups=grp,
                ins=[warm_in.opt()], outs=[warm_out.opt()],
            )
            nc.gpsimd.dma_start(warm_back[:], warm_out[0:1, :])

            # ---------- prelude: small constants ----------
            ones = sb.tile([128, 32], f32, name="ones")
            nc.vector.memset(ones[:], 1.0)
            u_bc = sb.tile([128, H], f32, name="u_bc")
            nc.sync.dma_start(u_bc[:], u_p.ap().broadcast_to([128, H]))
            c0_sb = []
            for l in range(L):
                c0_l = sb.tile([B, UPC], f32, name=f"c0_{l}")
                nc.sync.dma_start(c0_l[:], c0s[l])
                c0_sb.append(c0_l)

            # DRAM bounce buffers for collectives
            ag_in = [dram.tile([UPC, B], f32r, name=f"ag_in{l}") for l in range(L)]
            ag_out = [dram.tile([H, B], f32r, name=f"ag_out{l}") for l in range(L)]
            ctx_loc = dram.tile([BPC, H], f32r, name="ctx_loc")
            ctx_all = dram.tile([B, H], f32r, name="ctx_all")
            hid_in = dram.tile([UPC, B], f32r, name="hid_in")
            hid_out = dram.tile([H, B], f32r, name="hid_out")
            invd_dram = dram.tile([1, BPC], f32, name="invd_dram")

            # ----- LSTM layers software-pipelined with attention batches -----
            e_all, p_all = [], []
            for b in range(BPC):
                e_all.append(sb.tile([128, ST], f32, name=f"e_all{b}"))
                p_all.append(sb.tile([128, ST], f32r, name=f"p_all{b}"))
            scratch = sb.tile([128, H], f32, name="scratch")
            d_ps = ps.tile([1, BPC], f32, name="d_ps")

            def emit_lstm_layer(l):
                if l == 0:
                    wsrc = [(wih0T, j) for j in range((2 * E) // 128)] + [
                        (whh0T, j) for j in range(H // 128)
                    ]
                    lsrc = [(xT, j) for j in range((2 * E) // 128)] + [
                        (h0T, (0, j)) for j in range(H // 128)
                    ]
                else:
                    wsrc = [(wihrT, (l - 1, j)) for j in range(H // 128)] + [
                        (whhrT, (l - 1, j)) for j in range(H // 128)
                    ]
                    lsrc = [(ag_out[l - 1], j) for j in range(H // 128)] + [
                        (h0T, (l, j)) for j in range(H // 128)
                    ]
                kt = len(wsrc)

                def _dma_src(eng, dst, src_, idx):
                    if isinstance(idx, tuple):
                        li, j = idx
                        eng.dma_start(dst[:], src_[li, j * 128:(j + 1) * 128, :])
                    else:
                        eng.dma_start(dst[:], src_[idx * 128:(idx + 1) * 128, :])

                gp = ps.tile([B, 4 * UPC], f32, name="gp", tag="gp", bufs=1)
                for k in range(kt):
                    wt = sb.tile([128, 4 * UPC], f32r, name="lw", tag="lw", bufs=24)
                    _dma_src(nc.scalar, wt, *wsrc[k])
                    rt = sb.tile([128, B], f32r, name="rk", tag="rk", bufs=26)
                    _dma_src(nc.gpsimd, rt, *lsrc[k])
                    nc.tensor.matmul(
                        gp[:], rt[:], wt[:], start=(k == 0), stop=(k == kt - 1),
                    )
                bias_bc = sb.tile([B, 4 * UPC], f32, name="bias_bc", tag="bias_bc", bufs=2)
                bsrc = b0r if l == 0 else brr
                boff = 0 if l == 0 else (l - 1) * 4 * UPC
                nc.sync.dma_start(
                    bias_bc[:],
                    bsrc.ap().bitcast(f32)[0:1, boff:boff + 4 * UPC]
                    .broadcast_to([B, 4 * UPC]),
                )
                gpb = sb.tile([B, 4 * UPC], f32, name="gpb", tag="gpb", bufs=2)
                nc.vector.tensor_add(gpb[:], gp[:], bias_bc[:])

                s_i = sb.tile([B, UPC], f32, name="s_i", tag="s_i", bufs=2)
                s_f = sb.tile([B, UPC], f32, name="s_f", tag="s_f", bufs=2)
                s_o = sb.tile([B, UPC], f32, name="s_o", tag="s_o", bufs=2)
                tg = sb.tile([B, UPC], f32, name="tg", tag="tg", bufs=2)
                nc.scalar.activation(s_i[:], gpb[:, 0:UPC], AF.Sigmoid)
                nc.scalar.activation(s_f[:], gpb[:, UPC:2 * UPC], AF.Sigmoid)
                nc.scalar.activation(s_o[:], gpb[:, 3 * UPC:4 * UPC], AF.Sigmoid)
                nc.scalar.activation(tg[:], gpb[:, 2 * UPC:3 * UPC], AF.Tanh)

                c_new = sb.tile([B, UPC], f32, name="c_new", tag="c_new", bufs=2)
                m2 = sb.tile([B, UPC], f32, name="m2", tag="m2", bufs=2)
                tcn = sb.tile([B, UPC], f32, name="tcn", tag="tcn", bufs=2)
                h_new = sb.tile([B, UPC], f32r, name="h_new", tag="h_new", bufs=2)
                nc.vector.tensor_mul(c_new[:], s_f[:], c0_sb[l][:])
                nc.vector.tensor_mul(m2[:], s_i[:], tg[:])
                nc.vector.tensor_add(c_new[:], c_new[:], m2[:])
                nc.scalar.activation(tcn[:], c_new[:], AF.Tanh)
                nc.vector.tensor_mul(h_new[:], s_o[:], tcn[:])

                nc.gpsimd.dma_start(cn_o[l], c_new[:])
                nc.gpsimd.dma_start(hn_o[l], h_new[:])
                nc.gpsimd.dma_start(ag_in[l].rearrange("u b -> b u"), h_new[:])
                nc.gpsimd.collective_compute(
                    "AllGather", OP.bypass, replica_groups=grp,
                    ins=[ag_in[l].opt()], outs=[ag_out[l].opt()],
                )

            def emit_attn_batch(b):
                cpa = ps.tile([1, 512], f32, name="cpa", tag="cpa", bufs=1)
                cpb = ps.tile([1, 512], f32, name="cpb", tag="cpb", bufs=1)
                for t in range(ST):
                    et = sb.tile([128, H], f32r, name="enc_t", tag="enc_t", bufs=7)
                    nc.sync.dma_start(et[:], encs[b, t * 128:(t + 1) * 128, :])
                    nc.vector.scalar_tensor_tensor(
                        out=scratch[:], in0=et.bitcast(f32)[:], scalar=1.0,
                        in1=u_bc[:], op0=OP.mult, op1=OP.mult,
                        accum_out=e_all[b][:, t:t + 1],
                    )
                    nc.scalar.activation(
                        p_all[b][:, t:t + 1], e_all[b][:, t:t + 1], AF.Exp
                    )
                    nc.tensor.matmul(
                        cpa[:], p_all[b][:, t:t + 1], et[:, 0:512],
                        start=(t == 0), stop=(t == ST - 1),
                    )
                    nc.tensor.matmul(
                        cpb[:], p_all[b][:, t:t + 1], et[:, 512:1024],
                        start=(t == 0), stop=(t == ST - 1),
                    )
                d_col = sb.tile([128, 1], f32, name="d_col", tag="d_col", bufs=2)
                nc.vector.reduce_sum(
                    d_col[:], p_all[b].bitcast(f32)[:], axis=mybir.AxisListType.X
                )
                nc.tensor.matmul(
                    d_ps[0:1, b:b + 1], d_col[:], ones[:, 0:1], start=True, stop=True
                )
                db = sb.tile([1, 1], f32, name="db", tag="db", bufs=2)
                invb = sb.tile([1, 1], f32, name="invb", tag="invb", bufs=2)
                nc.vector.tensor_copy(db[:], d_ps[0:1, b:b + 1])
                nc.vector.reciprocal(invb[:], db[:])
                cx_sb = sb.tile([1, H], f32r, name="cx_sb", tag="cx_sb", bufs=2)
                nc.vector.tensor_scalar_mul(cx_sb[:, 0:512], cpa[:], invb[0:1, 0:1])
                nc.vector.tensor_scalar_mul(cx_sb[:, 512:1024], cpb[:], invb[0:1, 0:1])
                nc.gpsimd.dma_start(ctx_loc[b:b + 1, :], cx_sb[:])

            for i in range(4):
                emit_lstm_layer(i)
                emit_attn_batch(i)

            nc.gpsimd.collective_compute(
                "AllGather", OP.bypass, replica_groups=grp,
                ins=[ctx_loc.opt()], outs=[ctx_all.opt()],
            )

            # ---------- hidden = tanh(Wal @ [out; context; key_e] + bal) -------
            KT = (2 * H + E) // 128  # 24
            wal_tiles = []
            for k in range(KT):
                wt = sb.tile([128, UPC], f32r, name="walt", tag="walt", bufs=KT)
                nc.sync.dma_start(wt[:], walT[k * 128:(k + 1) * 128, :])
                wal_tiles.append(wt)
            hr_tiles = []
            for k in range(KT):
                rt = sb.tile([128, B], f32r, name="hrk", tag="rk", bufs=26)
                if k < 8:
                    nc.gpsimd.dma_start(rt[:], ag_out[L - 1][k * 128:(k + 1) * 128, :])
                elif k < 16:
                    j = k - 8
                    nc.gpsimd.dma_start(
                        rt[:],
                        ctx_all[:, j * 128:(j + 1) * 128].rearrange("b h -> h b"),
                    )
                else:
                    j = k - 16
                    nc.gpsimd.dma_start(rt[:], xT[E + j * 128:E + (j + 1) * 128, :])
                hr_tiles.append(rt)

            hid_ps = ps.tile([B, UPC], f32, name="hid_ps")
            for k in range(KT):
                nc.tensor.matmul(
                    hid_ps[:], hr_tiles[k][:], wal_tiles[k][:],
                    start=(k == 0), stop=(k == KT - 1),
                )
            hbias_bc = sb.tile([B, UPC], f32, name="hbias_bc")
            nc.sync.dma_start(
                hbias_bc[:],
                balr.ap().bitcast(f32)[0:1, :].broadcast_to([B, UPC]),
            )
            hid_pb = sb.tile([B, UPC], f32, name="hid_pb")
            nc.vector.tensor_add(hid_pb[:], hid_ps[:], hbias_bc[:])
            hid_sb = sb.tile([B, UPC], f32r, name="hid_sb")
            nc.scalar.activation(hid_sb[:], hid_pb[:], AF.Tanh)
            nc.gpsimd.dma_start(hid_in.rearrange("u b -> b u"), hid_sb[:])
            nc.gpsimd.collective_compute(
                "AllGather", OP.bypass, replica_groups=grp,
                ins=[hid_in.opt()], outs=[hid_out.opt()],
            )
            hT_sb = sb.tile([128, 8 * B], f32r, name="hT_sb")
            for k in range(8):
                nc.gpsimd.dma_start(
                    hT_sb[:, k * B:(k + 1) * B], hid_out[k * 128:(k + 1) * 128, :]
                )

            # ---------- logits: vocab-sharded output projection (fp32r) -------
            n_ch = (VS + LGC - 1) // LGC
            for ch in range(n_ch):
                c0_ = ch * LGC
                cw = min(LGC, VS - c0_)
                lg = ps.tile([B, LGC], f32, name="lg", tag="lg", bufs=2)
                for k in range(8):
                    wt = sb.tile([128, LGC], f32r, name="wo", tag="wo", bufs=32)
                    nc.sync.dma_start(
                        wt[:, 0:cw], woutT[k * 128:(k + 1) * 128, c0_:c0_ + cw]
                    )
                    nc.tensor.matmul(
                        lg[:, 0:cw], hT_sb[:, k * B:(k + 1) * B], wt[:, 0:cw],
                        start=(k == 0), stop=(k == 7),
                    )
                bout_bc = sb.tile([B, LGC], f32, name="bout_bc", tag="bout_bc", bufs=2)
                nc.sync.dma_start(
                    bout_bc[:, 0:cw],
                    bout_p.ap().bitcast(f32)[0:1, c0_:c0_ + cw].broadcast_to([B, cw]),
                )
                lsb = sb.tile([B, LGC], f32, name="lsb", tag="lsb", bufs=3)
                nc.vector.tensor_add(lsb[:, 0:cw], lg[:, 0:cw], bout_bc[:, 0:cw])
                nc.sync.dma_start(logits_o[:, c0_:c0_ + cw], lsb[:, 0:cw])


            # ---------- attns outputs (off the critical path) ----------
            d_sb = sb.tile([1, BPC], f32, name="d_sb")
            inv_d = sb.tile([1, BPC], f32, name="inv_d")
            nc.vector.tensor_copy(d_sb[:], d_ps[:])
            nc.vector.reciprocal(inv_d[:], d_sb[:])
            nc.gpsimd.dma_start(invd_dram[:], inv_d[:])
            invd_bc = sb.tile([128, BPC], f32, name="invd_bc")
            nc.gpsimd.dma_start(invd_bc[:], invd_dram.broadcast_to([128, BPC]))
            for b in range(BPC):
                at_sb = sb.tile([128, ST], f32, name="at_sb", tag="at_sb", bufs=2)
                nc.vector.tensor_scalar_mul(
                    at_sb[:], p_all[b].bitcast(f32)[:], invd_bc[:, b:b + 1]
                )
                nc.gpsimd.dma_start(
                    attnsT_o[b].rearrange("(t p) -> p t", p=128), at_sb[:]
                )

    nc.finalize()
    return nc


def _host_prep(encoder_outputs, input_seq, keyword, h0, c0, emb,
               Wih0, Whh0, bih0, bhh0, Wih_r, Whh_r, bih_r, bhh_r,
               Wa, ba, v, Wal, bal, Wout, bout):
    f = np.float32
    enc = np.asarray(encoder_outputs, f)
    emb = np.asarray(emb, f)
    idx_sos = np.asarray(input_seq).astype(np.int64)[:, 0]
    idx_key = np.asarray(keyword).astype(np.int64)
    sos = emb[idx_sos]                      # [B, E]
    key_e = emb[idx_key]                    # [B, E]
    xT = np.ascontiguousarray(np.concatenate([sos, key_e], axis=1).T)  # [2E, B]

    Wa = np.asarray(Wa, f)
    v_ = np.asarray(v, f)
    u = (Wa[:, H:].T @ v_)[None, :].astype(f)  # [1, H]

    h0 = np.asarray(h0, f)
    c0 = np.asarray(c0, f)
    h0T = np.ascontiguousarray(h0.transpose(0, 2, 1))  # [L, H, B]

    Wih0 = np.asarray(Wih0, f).reshape(4, H, 2 * E)
    Whh0 = np.asarray(Whh0, f).reshape(4, H, H)
    Wih_r = np.asarray(Wih_r, f).reshape(L - 1, 4, H, H)
    Whh_r = np.asarray(Whh_r, f).reshape(L - 1, 4, H, H)
    b0_full = (np.asarray(bih0, f) + np.asarray(bhh0, f)).reshape(4, H)
    br_full = (np.asarray(bih_r, f) + np.asarray(bhh_r, f)).reshape(L - 1, 4, H)

    Wal = np.asarray(Wal, f)
    bal = np.asarray(bal, f)
    Wout = np.asarray(Wout, f)
    bout = np.asarray(bout, f)

    woutT_pad = np.zeros((H, VPAD), f)
    woutT_pad[:, :V] = Wout.T
    bout_pad = np.zeros(VPAD, f)
    bout_pad[:V] = bout

    in_maps = []
    for c in range(NC):
        sl = slice(c * UPC, (c + 1) * UPC)
        wih0T = np.ascontiguousarray(
            Wih0[:, sl, :].reshape(4 * UPC, 2 * E).T)        # [2E, 512]
        whh0T = np.ascontiguousarray(
            Whh0[:, sl, :].reshape(4 * UPC, H).T)            # [H, 512]
        wihrT = np.ascontiguousarray(
            Wih_r[:, :, sl, :].reshape(L - 1, 4 * UPC, H).transpose(0, 2, 1))
        whhrT = np.ascontiguousarray(
            Whh_r[:, :, sl, :].reshape(L - 1, 4 * UPC, H).transpose(0, 2, 1))
        in_maps.append({
            "encs": np.ascontiguousarray(enc[c * BPC:(c + 1) * BPC]),
            "xT": xT,
            "h0T": h0T,
            "c0s": np.ascontiguousarray(c0[:, :, sl]),               # [L, B, 128]
            "u": u,
            "wih0T": wih0T,
            "whh0T": whh0T,
            "wihrT": wihrT,
            "whhrT": whhrT,
            "b0r": np.ascontiguousarray(b0_full[:, sl].reshape(1, 4 * UPC)),
            "brr": np.ascontiguousarray(br_full[:, :, sl].reshape(1, (L - 1) * 4 * UPC)),
            "walT": np.ascontiguousarray(Wal[sl, :].T),              # [3072, 128]
            "balr": np.ascontiguousarray(bal[sl])[None, :],          # [1, 128]
            "woutT": np.ascontiguousarray(woutT_pad[:, c * VS:(c + 1) * VS]),
            "bout": np.ascontiguousarray(bout_pad[c * VS:(c + 1) * VS])[None, :],
        })
    return in_maps, key_e


def kernel(**inputs):
    global LAST_EXEC_TIME_NS
    from concourse.bass_utils import run_bass_kernel_spmd

    if "nc" not in _COMPILED:
        _COMPILED["nc"] = _build_nc()
    nc = _COMPILED["nc"]

    in_maps, _ = _host_prep(**inputs)

    trace = os.environ.get("KERNEL_TRACE", "0") == "1"
    res = run_bass_kernel_spmd(nc, in_maps, list(range(NC)), trace=trace)
    LAST_EXEC_TIME_NS = res.exec_time_ns
    _COMPILED["last_res"] = res

    r = res.results
    logits = np.concatenate([r[c]["logits_o"] for c in range(NC)], axis=1)[:, :V]
    hn = np.concatenate([r[c]["hn_o"] for c in range(NC)], axis=2)
    cn = np.concatenate([r[c]["cn_o"] for c in range(NC)], axis=2)
    attns = np.concatenate([r[c]["attnsT"] for c in range(NC)], axis=0)[:, None, :]
    return logits, hn, cn, attns
